# revision 31
# baseline (speedup 1.0000x reference)
"""Trainium2 Bass kernel for nn_DeformBlock (two RK4-integrated NODE blocks).

Sharding: pure data parallel over (batch, point-half): core c handles
batch b = c // 2 and points [(c % 2) * 2048, (c % 2 + 1) * 2048).
All MLP weights are replicated; the conditioning vectors
sf = tanh(code @ cond_w.T + cond_b) are precomputed on the host (tiny).

On-device layout is feature-major: activations live as [H-chunk(128), pts]
so every linear layer is lhsT = W.T chunk [K=128, M=128], rhs = act chunk
[K=128, N=512] with no transposes anywhere. Matmuls run in float32r
(1 cycle/row at N=512, i.e. bf16-rate fp32).
"""
import sys

sys.path.insert(0, '/opt/trn_rl_repo')

import numpy as np
import concourse.bass as bass
import concourse.tile as tile
from concourse import mybir
from concourse.bass_utils import run_bass_kernel_spmd

F32 = mybir.dt.float32
F32R = mybir.dt.float32r
AF = mybir.ActivationFunctionType
ALU = mybir.AluOpType

B, N, H, Z = 4, 4096, 512, 512
TIME, N_STEPS = 0.2, 4
DT = TIME / N_STEPS
NCORES = 8
NPTS = (B * N) // NCORES          # 2048 points per core
N_REPEAT = 1                      # timing-only knob: repeat the whole chain
HK = H // 128                     # 4 feature chunks
SL = 512                          # point slice (matmul free dim / PSUM bank)
NSL = NPTS // SL                  # 4 point slices


# --------------------------------------------------------------------------
# wait-split post-pass: this walrus build allows only ONE sync wait per
# instruction; Tile can emit more. Move excess waits onto NoOps inserted
# right before the over-limit instruction on the same engine.
# --------------------------------------------------------------------------
_noop_uid = [0]


def _noop_with_waits(engine, waits):
    _noop_uid[0] += 1
    n = mybir.InstNoOp(name=f"ws_noop_{_noop_uid[0]}", ins=[], outs=[], engine=engine)
    n.sync_info = mybir.SyncInfo(on_wait=list(waits), on_update=[])
    return n


def split_waits(nc, limit=1):
    for fn in nc.m.functions:
        for bb in fn.blocks:
            out, changed = [], False
            for inst in bb.instructions:
                si = inst.sync_info
                waits = list(si.on_wait) if si and si.on_wait else []
                if len(waits) > limit:
                    for w in waits[limit:]:
                        out.append(_noop_with_waits(inst.engine, [w]))
                    si.on_wait = waits[:limit]
                    inst.sync_info = si
                    changed = True
                out.append(inst)
            if changed:
                bb.instructions = out


# --------------------------------------------------------------------------
# kernel build
# --------------------------------------------------------------------------

def _emit_dyn(nc, sb, acts, psum, q, kout, W, post_slice=None):
    """Emit one dynamics evaluation: kout = dyn(q) for one f-block.

    q, kout: [3, NPTS] f32r state tiles. W: dict of SBUF const tiles.
    post_slice(n, ns) is invoked right after slice n's tanh so the caller
    can chain per-slice state math that overlaps the rest of l4.
    """
    w1, w2, w3, w4 = W["w1"], W["w2"], W["w3"], W["w4"]
    b1, b2, b3, b4, sf = W["b1"], W["b2"], W["b3"], W["b4"], W["sf"]

    # ---- l1 + gate: h = relu(W1 @ q + b1) * sf ----
    # relu split ACT (m 0-1) / DVE (m 2-3); gate always DVE (ts runs 2x fp32)
    h = acts.tile([128, HK, NPTS], F32R, tag="act")
    for m in range(HK):
        for n in range(NSL):
            ns = slice(n * SL, (n + 1) * SL)
            ps = psum.tile([128, SL], F32, tag="ps")
            nc.tensor.matmul(ps[:, :], w1[:, m * 128:(m + 1) * 128], q[:, ns],
                             start=True, stop=True)
            if m < 3:
                nc.scalar.activation(h[:, m, ns], ps[:, :], AF.Relu,
                                     bias=b1[:, m:m + 1])
            else:
                nc.vector.tensor_scalar(h[:, m, ns], ps[:, :], b1[:, m:m + 1],
                                        0.0, ALU.add, ALU.max)
            nc.vector.tensor_scalar_mul(h[:, m, ns], h[:, m, ns], sf[:, m:m + 1])

    # ---- l2 / l3: h' = relu(W @ h + b) + h ----
    # relu on ACT; residual adds on DVE (Pool's 1.1us adds would gate the
    # next layer's matmul stream — sub-700ns cadence needed here)
    hin = h
    for li, (w, b_) in enumerate(((w2, b2), (w3, b3))):
        add_eng = nc.vector
        hout = acts.tile([128, HK, NPTS], F32R, tag="act")
        for m in range(HK):
            pss = [psum.tile([128, SL], F32, tag="ps", name=f"ps_{m}_{n}")
                   for n in range(NSL)]
            for k in range(HK):
                for n in range(NSL):
                    ns = slice(n * SL, (n + 1) * SL)
                    nc.tensor.matmul(pss[n][:, :],
                                     w[:, k, m * 128:(m + 1) * 128],
                                     hin[:, k, ns],
                                     start=(k == 0), stop=(k == HK - 1))
            for n in range(NSL):
                ns = slice(n * SL, (n + 1) * SL)
                nc.scalar.activation(hout[:, m, ns], pss[n][:, :], AF.Relu,
                                     bias=b_[:, m:m + 1])
                add_eng.tensor_tensor(hout[:, m, ns], hout[:, m, ns],
                                      hin[:, m, ns], op=ALU.add)
        hin = hout

    # ---- l4: kout = tanh(W4 @ h + b4) ----
    # n-outer: slice n's psum closes after its 4 matmuls so the
    # tanh/axpy/next-l1 chain for early slices overlaps the rest of l4.
    for n in range(NSL):
        ns = slice(n * SL, (n + 1) * SL)
        ps4 = psum.tile([3, SL], F32, tag="ps", name=f"ps4_{n}")
        for k in range(HK):
            nc.tensor.matmul(ps4[:, :], w4[:, k, :], hin[:, k, ns],
                             start=(k == 0), stop=(k == HK - 1))
        nc.scalar.activation(kout[:, ns], ps4[:, :], AF.Tanh, bias=b4[:, 0:1])
        if post_slice is not None:
            post_slice(n, ns)


def build_nc():
    nc = bass.Bass()

    xt = nc.dram_tensor("xt", [3, NPTS], F32R, kind="ExternalInput")
    yt = nc.dram_tensor("yt", [3, NPTS], F32R, kind="ExternalOutput")
    dram = {}
    for f in ("f1", "f2"):
        dram[f] = {
            "w1": nc.dram_tensor(f + "_w1t", [3, H], F32R, kind="ExternalInput"),
            "w2": nc.dram_tensor(f + "_w2t", [H, H], F32R, kind="ExternalInput"),
            "w3": nc.dram_tensor(f + "_w3t", [H, H], F32R, kind="ExternalInput"),
            "w4": nc.dram_tensor(f + "_w4t", [H, 3], F32R, kind="ExternalInput"),
            "b1": nc.dram_tensor(f + "_b1", [128, HK], F32, kind="ExternalInput"),
            "b2": nc.dram_tensor(f + "_b2", [128, HK], F32, kind="ExternalInput"),
            "b3": nc.dram_tensor(f + "_b3", [128, HK], F32, kind="ExternalInput"),
            "b4": nc.dram_tensor(f + "_b4", [3, 1], F32, kind="ExternalInput"),
            "sf": nc.dram_tensor(f + "_sf", [128, HK], F32, kind="ExternalInput"),
        }

    with tile.TileContext(nc) as tc:
        with tc.tile_pool(name="consts", bufs=1) as consts, \
             tc.tile_pool(name="acts", bufs=2) as acts, \
             tc.tile_pool(name="states", bufs=1) as states, \
             tc.tile_pool(name="psum", bufs=8, space="PSUM") as psum:

            # x + small f1 tensors first so the first dyn can start while
            # the bulk 512x512 weights stream in.
            p = states.tile([3, NPTS], F32R, tag="p")
            nc.sync.dma_start(out=p, in_=xt[:, :])

            W = {}
            for f in ("f1", "f2"):
                d = dram[f]
                ws = {}
                w1 = consts.tile([3, H], F32R, tag=f + "w1", name=f + "w1")
                nc.sync.dma_start(out=w1, in_=d["w1"][:, :])
                ws["w1"] = w1
                for nm in ("b1", "b2", "b3", "sf"):
                    t = consts.tile([128, HK], F32, tag=f + nm, name=f + nm)
                    nc.sync.dma_start(out=t, in_=d[nm][:, :])
                    ws[nm] = t
                b4 = consts.tile([3, 1], F32, tag=f + "b4", name=f + "b4")
                nc.sync.dma_start(out=b4, in_=d["b4"][:, :])
                ws["b4"] = b4
                W[f] = ws
            for f in ("f1", "f2"):
                d = dram[f]
                w2 = consts.tile([128, HK, H], F32R, tag=f + "w2", name=f + "w2")
                w3 = consts.tile([128, HK, H], F32R, tag=f + "w3", name=f + "w3")
                for k in range(HK):
                    nc.sync.dma_start(out=w2[:, k, :], in_=d["w2"][k * 128:(k + 1) * 128, :])
                for k in range(HK):
                    nc.sync.dma_start(out=w3[:, k, :], in_=d["w3"][k * 128:(k + 1) * 128, :])
                w4 = consts.tile([128, HK, 3], F32R, tag=f + "w4", name=f + "w4")
                for k in range(HK):
                    nc.sync.dma_start(out=w4[:, k, :], in_=d["w4"][k * 128:(k + 1) * 128, :])
                W[f].update({"w2": w2, "w3": w3, "w4": w4})

            ks = {}
            for f in ("f1", "f2") * N_REPEAT:
                for step in range(N_STEPS):
                    # RK4 with an incrementally built combine accumulator:
                    # racc = p + (DT/6)k1 + (DT/3)k2 + (DT/3)k3, each term
                    # added right after its k is produced (off critical path);
                    # after k4 only ts+tt per slice remain before p'.
                    k1 = states.tile([3, NPTS], F32R, tag="k1")
                    qa = states.tile([3, NPTS], F32R, tag="tmp", bufs=3, name="qa")
                    racc = states.tile([3, NPTS], F32R, tag="racc")
                    t = states.tile([3, NPTS], F32R, tag="tmp", bufs=3, name="t")

                    def after_k1(n, ns):
                        # qa = p + (DT/2) k1 (critical: feeds dyn2's l1)
                        nc.vector.tensor_scalar_mul(qa[:, ns], k1[:, ns], DT / 2)
                        nc.vector.tensor_tensor(qa[:, ns], qa[:, ns], p[:, ns], op=ALU.add)
                        # racc = p + (DT/6) k1 (lazy, idle Pool engine)
                        nc.gpsimd.tensor_scalar_mul(racc[:, ns], k1[:, ns], DT / 6.0)
                        nc.gpsimd.tensor_tensor(racc[:, ns], racc[:, ns], p[:, ns], op=ALU.add)

                    _emit_dyn(nc, consts, acts, psum, p, k1, W[f], after_k1)

                    k2 = states.tile([3, NPTS], F32R, tag="k2")
                    qb = states.tile([3, NPTS], F32R, tag="tmp", bufs=3, name="qb")

                    def after_k2(n, ns):
                        nc.vector.tensor_scalar_mul(qb[:, ns], k2[:, ns], DT / 2)
                        nc.vector.tensor_tensor(qb[:, ns], qb[:, ns], p[:, ns], op=ALU.add)
                        nc.gpsimd.tensor_scalar_mul(t[:, ns], k2[:, ns], DT / 3.0)
                        nc.gpsimd.tensor_tensor(racc[:, ns], racc[:, ns], t[:, ns], op=ALU.add)

                    _emit_dyn(nc, consts, acts, psum, qa, k2, W[f], after_k2)

                    k3 = states.tile([3, NPTS], F32R, tag="k3")
                    qc = states.tile([3, NPTS], F32R, tag="tmp", bufs=3, name="qc")
                    t2 = states.tile([3, NPTS], F32R, tag="tmp", bufs=3, name="t2")

                    def after_k3(n, ns):
                        nc.vector.tensor_scalar_mul(qc[:, ns], k3[:, ns], DT)
                        nc.vector.tensor_tensor(qc[:, ns], qc[:, ns], p[:, ns], op=ALU.add)
                        nc.gpsimd.tensor_scalar_mul(t2[:, ns], k3[:, ns], DT / 3.0)
                        nc.gpsimd.tensor_tensor(racc[:, ns], racc[:, ns], t2[:, ns], op=ALU.add)

                    _emit_dyn(nc, consts, acts, psum, qb, k3, W[f], after_k3)

                    k4 = states.tile([3, NPTS], F32R, tag="k4")

                    def after_k4(n, ns):
                        nc.vector.tensor_scalar_mul(k4[:, ns], k4[:, ns], DT / 6.0)
                        nc.vector.tensor_tensor(p[:, ns], racc[:, ns], k4[:, ns], op=ALU.add)

                    _emit_dyn(nc, consts, acts, psum, qc, k4, W[f], after_k4)

            nc.sync.dma_start(out=yt[:, :], in_=p[:, :])

    split_waits(nc)
    return nc


# --------------------------------------------------------------------------
# host side
# --------------------------------------------------------------------------
_NC_CACHE = {}


def _get_nc():
    if "nc" not in _NC_CACHE:
        _NC_CACHE["nc"] = build_nc()
    return _NC_CACHE["nc"]


def _pack_bias(b):
    # [512] -> [128, 4] chunk-major columns
    return np.ascontiguousarray(b.reshape(HK, 128).T.astype(np.float32))


def _prep_in_maps(inputs):
    f = {k: np.asarray(v, dtype=np.float32) for k, v in inputs.items()}
    shared = {}
    for blk in ("f1", "f2"):
        shared[blk + "_w1t"] = np.ascontiguousarray(f[blk + "_l1_w"].T)   # [3, H]
        shared[blk + "_w2t"] = np.ascontiguousarray(f[blk + "_l2_w"].T)   # [H, H]
        shared[blk + "_w3t"] = np.ascontiguousarray(f[blk + "_l3_w"].T)   # [H, H]
        shared[blk + "_w4t"] = np.ascontiguousarray(f[blk + "_l4_w"].T)   # [H, 3]
        shared[blk + "_b1"] = _pack_bias(f[blk + "_l1_b"])
        shared[blk + "_b2"] = _pack_bias(f[blk + "_l2_b"])
        shared[blk + "_b3"] = _pack_bias(f[blk + "_l3_b"])
        shared[blk + "_b4"] = np.ascontiguousarray(
            f[blk + "_l4_b"].reshape(3, 1).astype(np.float32))

    code = f["code"]  # [B, 1, Z]
    sf = {}
    for blk in ("f1", "f2"):
        s = np.tanh(code[:, 0, :] @ f[blk + "_cond_w"].T + f[blk + "_cond_b"])
        sf[blk] = s.astype(np.float32)  # [B, H]

    x = f["x"]  # [B, N, 3]
    in_maps = []
    for c in range(NCORES):
        b, half = divmod(c, 2)
        xs = x[b, half * NPTS:(half + 1) * NPTS, :]  # [NPTS, 3]
        m = dict(shared)
        m["xt"] = np.ascontiguousarray(xs.T)          # [3, NPTS]
        m["f1_sf"] = _pack_bias(sf["f1"][b])
        m["f2_sf"] = _pack_bias(sf["f2"][b])
        in_maps.append(m)
    return in_maps


def kernel(**inputs) -> np.ndarray:
    nc = _get_nc()
    in_maps = _prep_in_maps(inputs)
    res = run_bass_kernel_spmd(nc, in_maps, core_ids=list(range(NCORES)))
    y = np.empty((B, N, 3), dtype=np.float32)
    for c in range(NCORES):
        b, half = divmod(c, 2)
        y[b, half * NPTS:(half + 1) * NPTS, :] = res.results[c]["yt"].T
    return y
